# revision 4
# baseline (speedup 1.0000x reference)
"""Trainium2 Bass kernel for mixed softmax + relu^2 attention.

Reference computation (B=4, S=2048, D=768, H=12, DH=64):
    q = split_heads(hidden @ Wq.T + bq)        # [B,H,S,DH]
    k = split_heads(hidden @ Wk.T + bk)
    v = split_heads(hidden @ Wv.T + bv)
    scores = q @ k.T / sqrt(DH)                # [B,H,S,S]
    attn = m0 * softmax(scores) + m1 * relu(scores)^2,  (m0,m1) = softmax(w_mix)
    out = merge_heads(attn @ v) @ Wo.T + bo

Sharding over 8 NeuronCores: core = (batch b = core//2, head-group g = core%2 of
6 heads).  Each core computes its 6 heads' full SxS attention and a partial
output projection over its 384 context dims; the host sums the two partials
per batch.

v2 pipeline (per head pair p, q-chunk qc; k on partitions):
  - scoresT tile [k=128, 2*512] = KT.T @ QT, both heads side by side (the two
    matmuls land on row-strips (0,0)/(64,0) so they pack in the PE array).
  - e = exp(scoresT) on ACT; r = relu(scoresT)^2 on DVE (custom op) - the two
    PSUM->SBUF elementwise passes, one per engine.
  - e-AV: [m0*V | ones].T @ e -> pse (row 64 = softmax denominator Z);
    r-AV: (m1*V).T @ r -> psr, col-strip packed (0,0)/(0,64).
  - combine (off the two hot engines): ACT evicts pse/psr -> bf16 SBUF (frees
    the PSUM banks for the next chunk), DVE computes 1/Z, GpSimd broadcasts
    1/Z and does ctx = e_ctx * (1/Z) + r_ctx.
  - Q/K projection and out-projection PSUM groups are spread as "filler
    bursts" through every k-loop, and small always-ready warmer matmuls
    bridge the PE's dependency stalls: the PE stream then has no idle
    windows, which keeps the HAM clock-gate at 2.4 GHz (otherwise the PE
    oscillates down to 1.2 GHz for most of the attention phase).

softmax(w_mix) and 1/sqrt(DH) are compile-time constants folded into
activation scales / V eviction scales.  Zero biases skip the bias path; if
biases are nonzero they are folded in via an augmented (ones-row) contraction
k-tile.
"""

from contextlib import ExitStack

import numpy as np
import ml_dtypes

import concourse.bass as bass
import concourse.mybir as mybir
import concourse.tile as tile
from concourse import bacc, dve_ops
from concourse.bass_utils import run_bass_kernel_spmd
from concourse.dve_spec import Spec, Src0, relu as _sp_relu, sq as _sp_sq


def _register_relu_sq():
    """Custom fused DVE op: out = relu(in0)^2 in a single pass."""
    for op in dve_ops.OPS:
        if op.name == "RELU_SQ_ANT":
            return op
    op = dve_ops.DveOp(
        "RELU_SQ_ANT",
        Spec(body=_sp_sq(_sp_relu(Src0)),
             reference=lambda in0: np.maximum(in0, 0.0) ** 2),
        subdim=False,
        uops_sha={"v3": "8abca05ebc329c1b", "v4": "4b83c053374efcdc"},
    )
    dve_ops.OPS.append(op)
    dve_ops.CUSTOM_DVE_SPECS[op.name] = op.spec
    dve_ops._SUB_OPCODE_FOR_NAME[op.name] = (
        dve_ops._CUSTOM_DVE_ROW_BASE + len(dve_ops.OPS) - 1
    )
    return op


RELU_SQ = _register_relu_sq()

B, S, D, H, DH = 4, 2048, 768, 12, 64
NCORES = 8
HL = H // 2          # local heads per core = 6
HPAIRS = HL // 2     # head pairs = 3
DLOC = HL * DH       # local context dims = 384
KTILES = S // 128    # 16
QCHUNK = 512
NQC = S // QCHUNK    # 4
DKT = D // 128       # 6 contraction tiles for projections

F32 = mybir.dt.float32
BF16 = mybir.dt.bfloat16
NP_BF16 = ml_dtypes.bfloat16
AF = mybir.ActivationFunctionType
OP = mybir.AluOpType

# AV matmuls consume elementwise results this many k-tiles behind the scores
# matmul, so the in-order PE stream never waits on the elementwise chain.
import os as _os
AV_DELAY = int(_os.environ.get("AV_DELAY", "5"))
# insert an always-ready "HAM warmer" matmul before the scores of every
# WARM_EVERY'th k-tile (0 disables).  These bridge the PE's dependency stalls
# so the HAM clock-gate never sees an idle window and the PE stays at 2.4GHz.
WARM_EVERY = int(_os.environ.get("WARM_EVERY", "2"))
# out-projection for chunk i is emitted after this many k-tiles of chunk i+1
# so its PSUM tiles and ctx inputs are long since ready when the PE reaches it
OUT_DELAY_KT = 6

_KERNEL_CACHE: dict = {}


def build_kernel(m0: float, m1: float, has_bias: bool, repeat: int = 1):
    nc = bacc.Bacc("TRN2", target_bir_lowering=False, debug=False)

    hT = nc.dram_tensor("hT", [D, S], BF16, kind="ExternalInput").ap()
    wqT = nc.dram_tensor("wqT", [D, DLOC], BF16, kind="ExternalInput").ap()
    wkT = nc.dram_tensor("wkT", [D, DLOC], BF16, kind="ExternalInput").ap()
    wvT = nc.dram_tensor("wvT", [D, DLOC], BF16, kind="ExternalInput").ap()
    woT = nc.dram_tensor("woT", [DLOC, D], BF16, kind="ExternalInput").ap()
    if has_bias:
        hb = nc.dram_tensor("hb", [1, S], BF16, kind="ExternalInput").ap()
        wqb = nc.dram_tensor("wqb", [1, DLOC], BF16, kind="ExternalInput").ap()
        wkb = nc.dram_tensor("wkb", [1, DLOC], BF16, kind="ExternalInput").ap()
        wvb = nc.dram_tensor("wvb", [1, DLOC], BF16, kind="ExternalInput").ap()
    out = nc.dram_tensor("out", [D, S], F32, kind="ExternalOutput").ap()

    qk_scale = 1.0 / float(np.sqrt(DH))

    with tile.TileContext(nc) as tc, ExitStack() as ctx:
        # ---------------- persistent SBUF ----------------
        pp = ctx.enter_context(tc.tile_pool(name="persist", bufs=1))

        h_t = [pp.tile([128, S], BF16, tag=f"ht{k}", name=f"ht{k}") for k in range(DKT)]
        wq_t = [pp.tile([128, DLOC], BF16, tag=f"wq{k}", name=f"wq{k}") for k in range(DKT)]
        wk_t = [pp.tile([128, DLOC], BF16, tag=f"wk{k}", name=f"wk{k}") for k in range(DKT)]
        wv_t = [pp.tile([128, DLOC], BF16, tag=f"wv{k}", name=f"wv{k}") for k in range(DKT)]
        wo_t = [pp.tile([128, D], BF16, tag=f"wo{c}", name=f"wo{c}") for c in range(HPAIRS)]
        for k in range(DKT):
            nc.sync.dma_start(h_t[k][:], hT[k * 128:(k + 1) * 128, :])
            nc.sync.dma_start(wq_t[k][:], wqT[k * 128:(k + 1) * 128, :])
            nc.sync.dma_start(wk_t[k][:], wkT[k * 128:(k + 1) * 128, :])
            nc.sync.dma_start(wv_t[k][:], wvT[k * 128:(k + 1) * 128, :])
        for c in range(HPAIRS):
            nc.sync.dma_start(wo_t[c][:], woT[c * 128:(c + 1) * 128, :])
        if has_bias:
            hb_t = pp.tile([1, S], BF16, tag="hbt")
            wqb_t = pp.tile([1, DLOC], BF16, tag="wqbt")
            wkb_t = pp.tile([1, DLOC], BF16, tag="wkbt")
            wvb_t = pp.tile([1, DLOC], BF16, tag="wvbt")
            nc.sync.dma_start(hb_t[:], hb[:, :])
            nc.sync.dma_start(wqb_t[:], wqb[:, :])
            nc.sync.dma_start(wkb_t[:], wkb[:, :])
            nc.sync.dma_start(wvb_t[:], wvb[:, :])

        qt_s = [pp.tile([128, S], BF16, tag=f"qt{p}", name=f"qt{p}") for p in range(HPAIRS)]
        kt_s = [pp.tile([128, S], BF16, tag=f"kt{p}", name=f"kt{p}") for p in range(HPAIRS)]
        # V with ones column per head (cols 65a..65a+63 = m0*V, col 65a+64 = 1)
        v1_s = [pp.tile([128, HL * (DH + 1)], BF16, tag=f"v1{t}", name=f"v1{t}") for t in range(KTILES)]
        # V scaled by m1 for the relu^2 branch
        v2_s = [pp.tile([128, DLOC], BF16, tag=f"v2{t}", name=f"v2{t}") for t in range(KTILES)]
        ctx_s = [pp.tile([128, S], BF16, tag=f"cx{p}", name=f"cx{p}") for p in range(HPAIRS)]

        nkt = DKT + (1 if has_bias else 0)

        def proj_lhs(w_t, w_b, k, p):
            if k < DKT:
                return w_t[k][:, p * 128:(p + 1) * 128]
            return w_b[:, p * 128:(p + 1) * 128]

        def phases(first: bool, last: bool):
            # ---------------- phase 1: V projection (+ pair-0 Q/K on rep 0) ----
            def proj_group(pool, p, qc, which, tag):
                """One Q- or K-projection accumulation group (6 matmuls + DVE
                eviction) for head pair p / q-chunk qc, in a scores-pool slot.
                Emitted as side-work bursts that keep the PE stream dense."""
                cols = bass.ts(qc, QCHUNK)
                ps = pool.tile([128, QCHUNK], F32, tag=tag)
                w_t, w_b = (wq_t, has_bias and wqb_t) if which == "q" else \
                           (wk_t, has_bias and wkb_t)
                for k in range(nkt):
                    rhs = h_t[k][:, cols] if k < DKT else hb_t[:, cols]
                    nc.tensor.matmul(ps[:], proj_lhs(w_t, w_b, k, p), rhs,
                                     start=(k == 0), stop=(k == nkt - 1))
                if which == "q":
                    # fold 1/sqrt(DH) into Q
                    nc.vector.tensor_scalar_mul(qt_s[p][:, cols], ps[:], qk_scale)
                else:
                    nc.vector.tensor_copy(kt_s[p][:, cols], ps[:])

            def qk_proj_qc(pool, p, qc, tq="q", tk="k"):
                proj_group(pool, p, qc, "q", tq)
                proj_group(pool, p, qc, "k", tk)

            with tc.tile_pool(name="p1ps", bufs=2, space="PSUM") as p1ps, \
                 tc.tile_pool(name="p1v", bufs=2, space="PSUM") as p1vps:
                for t in range(KTILES):
                    rows = bass.ts(t, 128)
                    psv = p1vps.tile([128, DLOC], F32, tag="v")
                    for k in range(nkt):
                        lhsT = h_t[k][:, rows] if k < DKT else hb_t[:, rows]
                        rhs = wv_t[k][:] if k < DKT else wvb_t[:]
                        nc.tensor.matmul(psv[:], lhsT, rhs, start=(k == 0), stop=(k == nkt - 1))
                    v1_3d = v1_s[t][:, :].rearrange("p (a d) -> p a d", d=DH + 1)
                    psv_3d = psv[:, :].rearrange("p (a d) -> p a d", d=DH)
                    nc.scalar.activation(v1_3d[:, :, 0:DH], psv_3d[:, :, :], AF.Copy, scale=m0)
                    nc.scalar.activation(v2_s[t][:], psv[:], AF.Copy, scale=m1)
                    nc.gpsimd.memset(v1_3d[:, :, DH:DH + 1], 1.0)
                if first:
                    for qc in range(NQC):
                        qk_proj_qc(p1ps, 0, qc)

            # ---------------- phase 2: attention ----------------
            # A global queue of PE "filler" bursts (next pair's Q/K projection
            # groups, previous chunk's out-projection groups) is drained at
            # fixed k-tile positions of EVERY k-loop.  The PE stream then has
            # no idle windows, which keeps the HAM clock-gate at full rate
            # (K=8/8) through the whole attention phase.
            with tc.tile_pool(name="scps", bufs=2, space="PSUM") as scps, \
                 tc.tile_pool(name="acps", bufs=1, space="PSUM") as acps, \
                 tc.tile_pool(name="ewsb", bufs=AV_DELAY + 2) as ewsb, \
                 tc.tile_pool(name="cbsb", bufs=2) as cbsb:

                side_q = []

                def warm_mm():
                    """Dependency-free matmul into the pso slot: pure HAM
                    warmer, result never read."""
                    ps = acps.tile([128, QCHUNK], F32, tag="pso", name="warm")
                    nc.tensor.matmul(ps[:], wo_t[0][:, 0:128], h_t[0][:, 0:QCHUNK],
                                     start=True, stop=True)

                def out_proj_group(qc, ot):
                    """One out-projection accumulation group: 3 matmuls over
                    the head pairs' ctx + ACT eviction + DMA."""
                    cols = bass.ts(qc, QCHUNK)
                    pso = acps.tile([128, QCHUNK], F32, tag="pso",
                                    name=f"pso{qc}_{ot}")
                    orows = bass.ts(ot, 128)
                    for c in range(HPAIRS):
                        nc.tensor.matmul(pso[:], wo_t[c][:, orows],
                                         ctx_s[c][:, cols],
                                         start=(c == 0), stop=(c == HPAIRS - 1))
                    ob = cbsb.tile([128, QCHUNK], F32, tag="ob")
                    nc.scalar.activation(ob[:], pso[:], AF.Copy)
                    nc.sync.dma_start(out[ot * 128:(ot + 1) * 128, cols], ob[:])

                for p in range(HPAIRS):
                    # enqueue the next pair's (or next repeat's pair-0) Q/K
                    # projection groups as filler bursts
                    if p < HPAIRS - 1:
                        for qc_ in range(NQC):
                            for w_ in ("q", "k"):
                                side_q.append((lambda pp_, qc2, w2: lambda:
                                    proj_group(scps, pp_, qc2, w2,
                                               "sa" if w2 == "q" else "sb"))
                                    (p + 1, qc_, w_))
                    elif not last:
                        for qc_ in range(NQC):
                            for w_ in ("q", "k"):
                                side_q.append((lambda qc2, w2: lambda:
                                    proj_group(scps, 0, qc2, w2,
                                               "sa" if w2 == "q" else "sb"))
                                    (qc_, w_))
                    a0, a1 = 2 * p, 2 * p + 1
                    for qc in range(NQC):
                        cols = bass.ts(qc, QCHUNK)
                        pse_a = acps.tile([128, QCHUNK], F32, tag="peA")
                        pse_b = acps.tile([128, QCHUNK], F32, tag="peB")
                        psr = acps.tile([128, QCHUNK], F32, tag="pr")
                        pending = {}

                        def av_mms(t):
                            eta, etb, rta, rtb = pending.pop(t)
                            st, sp = t == 0, t == KTILES - 1
                            va = v1_s[t][:, a0 * (DH + 1):(a0 + 1) * (DH + 1)]
                            vb = v1_s[t][:, a1 * (DH + 1):(a1 + 1) * (DH + 1)]
                            nc.tensor.matmul(pse_a[0:DH + 1, :], va, eta[:],
                                             start=st, stop=sp)
                            nc.tensor.matmul(pse_b[0:DH + 1, :], vb, etb[:],
                                             start=st, stop=sp)
                            nc.tensor.matmul(psr[0:64, :], v2_s[t][:, a0 * DH:(a0 + 1) * DH],
                                             rta[:], start=st, stop=sp)
                            nc.tensor.matmul(psr[64:128, :], v2_s[t][:, a1 * DH:(a1 + 1) * DH],
                                             rtb[:], start=st, stop=sp)

                        for t in range(KTILES):
                            krows = bass.ts(t, 128)
                            # bridge PE dependency stalls with an always-ready
                            # matmul so the HAM clock-gate stays at 2.4 GHz
                            if WARM_EVERY and t % WARM_EVERY == 0 and not (
                                    t in (1, 4, 7, 10, 13) and side_q):
                                warm_mm()
                            # per-head 1-bank score tiles: elementwise on head a
                            # can start as soon as its matmul lands, and the
                            # WAR slot rotation is finer-grained
                            ssa = scps.tile([128, QCHUNK], F32, tag="sa")
                            ssb = scps.tile([128, QCHUNK], F32, tag="sb")
                            nc.tensor.matmul(ssa[:], kt_s[p][0:64, krows],
                                             qt_s[p][0:64, cols])
                            nc.tensor.matmul(ssb[:], kt_s[p][64:128, krows],
                                             qt_s[p][64:128, cols])

                            eta = ewsb.tile([128, QCHUNK], BF16, tag="ea")
                            etb = ewsb.tile([128, QCHUNK], BF16, tag="eb")
                            rta = ewsb.tile([128, QCHUNK], BF16, tag="ra")
                            rtb = ewsb.tile([128, QCHUNK], BF16, tag="rb")
                            nc.scalar.activation(eta[:], ssa[:], AF.Exp)
                            nc.vector._custom_dve(RELU_SQ, out=rta[:], in0=ssa[:])
                            nc.scalar.activation(etb[:], ssb[:], AF.Exp)
                            nc.vector._custom_dve(RELU_SQ, out=rtb[:], in0=ssb[:])
                            pending[t] = (eta, etb, rta, rtb)
                            if t >= AV_DELAY:
                                av_mms(t - AV_DELAY)
                            if t in (1, 4, 7, 10, 13) and side_q:
                                side_q.pop(0)()
                        for t in range(KTILES - AV_DELAY, KTILES):
                            av_mms(t)

                        # ---- combine: ctx = e_ctx * (1/Z) + r_ctx ----
                        # Z rows -> SBUF (ACT), one reciprocal + one GpSimd
                        # broadcast covering both heads ([.. | ..] halves).
                        zrow = cbsb.tile([1, 2 * QCHUNK], F32, tag="zrow")
                        nc.scalar.activation(zrow[0:1, 0:QCHUNK], pse_a[64:65, :], AF.Copy)
                        nc.scalar.activation(zrow[0:1, QCHUNK:2 * QCHUNK], pse_b[64:65, :], AF.Copy)
                        zrec = cbsb.tile([1, 2 * QCHUNK], F32, tag="zrec")
                        nc.vector.reciprocal_approx_fast(zrec[:], zrow[:])
                        zb = cbsb.tile([128, 2 * QCHUNK], F32, tag="zb")
                        nc.gpsimd.partition_broadcast(zb[:, :], zrec[0:1, :], channels=128)
                        prod = cbsb.tile([128, QCHUNK], BF16, tag="prod")
                        if p < HPAIRS - 1:
                            # off the hot engines: ACT evicts the accumulators
                            # (frees PSUM banks), GpSimd multiplies and adds.
                            cc = cbsb.tile([128, QCHUNK], BF16, tag="cc")
                            rr = cbsb.tile([128, QCHUNK], BF16, tag="rr")
                            nc.scalar.activation(cc[0:64, :], pse_a[0:64, :], AF.Copy)
                            nc.scalar.activation(cc[64:128, :], pse_b[0:64, :], AF.Copy)
                            nc.scalar.activation(rr[:, :], psr[:, :], AF.Copy)
                            nc.gpsimd.tensor_tensor(prod[0:64, :], cc[0:64, :],
                                                    zb[0:64, 0:QCHUNK], op=OP.mult)
                            nc.gpsimd.tensor_tensor(prod[64:128, :], cc[64:128, :],
                                                    zb[64:128, QCHUNK:2 * QCHUNK], op=OP.mult)
                            nc.gpsimd.tensor_tensor(ctx_s[p][:, cols], prod[:], rr[:], op=OP.add)
                        else:
                            # last pair feeds the out-projection directly below:
                            # short-latency DVE path straight from PSUM
                            nc.vector.tensor_tensor(prod[0:64, :], pse_a[0:64, :],
                                                    zb[0:64, 0:QCHUNK], op=OP.mult)
                            nc.vector.tensor_tensor(prod[64:128, :], pse_b[0:64, :],
                                                    zb[64:128, QCHUNK:2 * QCHUNK], op=OP.mult)
                            nc.vector.tensor_tensor(ctx_s[p][:, cols], prod[:], psr[:], op=OP.add)

                        # output projection for this q-chunk becomes filler
                        # bursts drained by subsequent k-loops
                        if p == HPAIRS - 1:
                            for ot in range(D // 128):
                                side_q.append((lambda qc2, ot2: lambda:
                                    out_proj_group(qc2, ot2))(qc, ot))

                # drain remaining filler bursts before the pools close
                while side_q:
                    side_q.pop(0)()

        for _rep in range(repeat):
            phases(first=(_rep == 0), last=(_rep == repeat - 1))

    nc.compile()
    return nc


def _get_kernel(m0: float, m1: float, has_bias: bool):
    key = (round(m0, 9), round(m1, 9), has_bias)
    if key not in _KERNEL_CACHE:
        _KERNEL_CACHE[key] = build_kernel(m0, m1, has_bias)
    return _KERNEL_CACHE[key]


def make_in_maps(inputs: dict) -> tuple[list[dict], float, float, bool]:
    hidden = np.asarray(inputs["hidden_states"], dtype=np.float32)
    Wq = np.asarray(inputs["Wq"], dtype=np.float32)
    Wk = np.asarray(inputs["Wk"], dtype=np.float32)
    Wv = np.asarray(inputs["Wv"], dtype=np.float32)
    Wo = np.asarray(inputs["Wo"], dtype=np.float32)
    bq = np.asarray(inputs["bq"], dtype=np.float32)
    bk = np.asarray(inputs["bk"], dtype=np.float32)
    bv = np.asarray(inputs["bv"], dtype=np.float32)
    w_mix = np.asarray(inputs["w_mix"], dtype=np.float32)

    e = np.exp(w_mix - w_mix.max())
    mix = e / e.sum()
    m0, m1 = float(mix[0]), float(mix[1])
    has_bias = bool(bq.any() or bk.any() or bv.any())

    def bf(x):
        return np.ascontiguousarray(x).astype(NP_BF16)

    in_maps = []
    for core in range(NCORES):
        b, g = core // 2, core % 2
        rows = slice(DLOC * g, DLOC * (g + 1))
        m = {
            "hT": bf(hidden[b].T),
            "wqT": bf(Wq[rows].T),
            "wkT": bf(Wk[rows].T),
            "wvT": bf(Wv[rows].T),
            "woT": bf(Wo[:, rows].T),
        }
        if has_bias:
            m["hb"] = bf(np.ones((1, S), dtype=np.float32))
            m["wqb"] = bf(bq[rows][None, :])
            m["wkb"] = bf(bk[rows][None, :])
            m["wvb"] = bf(bv[rows][None, :])
        in_maps.append(m)
    return in_maps, m0, m1, has_bias


def assemble_output(results: list[dict], bo: np.ndarray) -> np.ndarray:
    out = np.empty((B, S, D), dtype=np.float32)
    for b in range(B):
        out[b] = (results[2 * b]["out"] + results[2 * b + 1]["out"]).T
    if bo.any():
        out += bo
    return out


def _spot_check(out: np.ndarray, inputs: dict, rng: np.random.Generator) -> bool:
    """Recompute one random query row per batch on the host (covers all 8
    cores' partial outputs) and compare; guards against transient HW faults."""
    hidden = np.asarray(inputs["hidden_states"], dtype=np.float32)
    Wq = np.asarray(inputs["Wq"], dtype=np.float32)
    Wk = np.asarray(inputs["Wk"], dtype=np.float32)
    Wv = np.asarray(inputs["Wv"], dtype=np.float32)
    Wo = np.asarray(inputs["Wo"], dtype=np.float32)
    bq = np.asarray(inputs["bq"], dtype=np.float32)
    bk = np.asarray(inputs["bk"], dtype=np.float32)
    bv = np.asarray(inputs["bv"], dtype=np.float32)
    bo = np.asarray(inputs["bo"], dtype=np.float32)
    w_mix = np.asarray(inputs["w_mix"], dtype=np.float32)
    e = np.exp(w_mix - w_mix.max())
    m0, m1 = e / e.sum()
    for b in range(B):
        s = int(rng.integers(0, S))
        q = (hidden[b, s] @ Wq.T + bq).reshape(H, DH) / np.sqrt(DH)
        k = (hidden[b] @ Wk.T + bk).reshape(S, H, DH)
        v = (hidden[b] @ Wv.T + bv).reshape(S, H, DH)
        scores = np.einsum("hd,khd->hk", q, k)
        sm = np.exp(scores - scores.max(axis=1, keepdims=True))
        sm /= sm.sum(axis=1, keepdims=True)
        attn = m0 * sm + m1 * np.maximum(scores, 0.0) ** 2
        ctx = np.einsum("hk,khd->hd", attn, v).reshape(D)
        want = ctx @ Wo.T + bo
        got = out[b, s]
        rel = np.abs(got - want).max() / max(np.abs(want).max(), 1e-6)
        if not np.isfinite(got).all() or rel > 0.05:
            return False
    return True


def kernel(**inputs) -> np.ndarray:
    in_maps, m0, m1, has_bias = make_in_maps(inputs)
    nc = _get_kernel(m0, m1, has_bias)
    bo = np.asarray(inputs["bo"], dtype=np.float32)
    rng = np.random.default_rng(12345)
    out = None
    for _attempt in range(3):
        res = run_bass_kernel_spmd(nc, in_maps, core_ids=list(range(NCORES)))
        out = assemble_output(res.results, bo)
        if np.isfinite(out).all() and _spot_check(out, inputs, rng):
            return out
    return out
